# revision 17
# baseline (speedup 1.0000x reference)
"""Trainium2 Bass kernel for nn_BasicBlock (binary-conv residual block).

Math (reference):
  h  = BN3( RPReLU1(BN1(bconv(sign(x), w1))) + x )
  out= BN4( RPReLU2(BN2(bconv(sign(h), w2))) + h )
with training-mode BN over the FULL batch (exact cross-device stats),
bconv = conv3x3(pad=1) with weights sign(w)*mean(|w|) per out-channel.

Strategy: data-parallel over batch on 8 NeuronCores (16 images/core).
 - fp8e4 +-1 activations/weights; conv = 9 shifted DoubleRow matmuls
   (each contracts both 128-channel halves) into PSUM; integer sums exact.
 - alpha (mean|w|) folded into BN1/BN2 affine; constant per-channel shifts
   absorbed by downstream BNs are dropped.
 - Exact BN stats via per-channel-half (sum,sumsq) AllGather + local add;
   the first half's stats/combine hide under the second half's conv.
 - h' (pre-BN3, scaled by c3) round-trips through DRAM during conv2.
"""

import sys

import numpy as np

sys.path.insert(0, "/opt/trn_rl_repo")

from contextlib import ExitStack

import concourse.bacc as bacc
import concourse.bass as bass
import concourse.mybir as mybir
import concourse.tile as tile
from concourse.masks import make_identity

dt = mybir.dt
AF = mybir.ActivationFunctionType
ALU = mybir.AluOpType
AX = mybir.AxisListType

C = 256
H = W = 28
PH = PW = 30
SP = PH * PW          # padded pixels / image
HW = H * W            # valid pixels / image
MARG = 32             # margin around padded free axis (shifts up to +-31)
EPS = 1e-5
NPAR = 12
PJ = dict(g1=0, b1=1, g2=2, b2=3, g3=4, b3=5, g4=6, b4=7,
          gamma1=8, beta1=9, gamma2=10, beta2=11)


def _off(d):
    kh, kw = d // 3, d % 3
    return (kh - 1) * PW + (kw - 1)


def build_nc(n_img, n_cores):
    nc = bacc.Bacc("TRN2", target_bir_lowering=False, num_devices=n_cores,
                   name="basicblock")
    x_d = nc.declare_dram_parameter("x", [n_img, C, H, W], dt.float32, isOutput=False)
    w1_d = nc.declare_dram_parameter("w1", [C, C, 3, 3], dt.float32, isOutput=False)
    w2_d = nc.declare_dram_parameter("w2", [C, C, 3, 3], dt.float32, isOutput=False)
    p_d = nc.declare_dram_parameter("pars", [NPAR, C], dt.float32, isOutput=False)
    o_d = nc.declare_dram_parameter("out", [n_img, C, H, W], dt.float32, isOutput=True)

    FREE = n_img * SP
    XBW = FREE + 2 * MARG
    NLOC = float(n_img * HW)
    NTOT = float(n_cores * n_img * HW)
    rg = [list(range(n_cores))]

    with ExitStack() as ctx:
        tc = ctx.enter_context(tile.TileContext(nc))
        sing = ctx.enter_context(tc.tile_pool(name="sing", bufs=1))
        xbp = ctx.enter_context(tc.tile_pool(name="xbp", bufs=1))
        wtp = ctx.enter_context(tc.tile_pool(name="wtp", bufs=2))
        wop = ctx.enter_context(tc.tile_pool(name="wop", bufs=2))
        actp = ctx.enter_context(tc.tile_pool(name="actp", bufs=2 * n_img))
        chkp = ctx.enter_context(tc.tile_pool(name="chkp", bufs=4))
        tmpp = ctx.enter_context(tc.tile_pool(name="tmpp", bufs=3))
        stp = ctx.enter_context(tc.tile_pool(name="stp", bufs=1))
        psp = ctx.enter_context(tc.tile_pool(name="psp", bufs=8, space="PSUM"))
        dccp = ctx.enter_context(tc.tile_pool(name="dccp", bufs=1, space="DRAM"))
        dswp = ctx.enter_context(tc.tile_pool(name="dswp", bufs=2 * n_img, space="DRAM"))

        # ---- constants / params -------------------------------------------------
        ident = sing.tile([128, 128], dt.bfloat16, name="ident")
        make_identity(nc, ident)
        par = sing.tile([128, NPAR, 2], dt.float32, name="par")
        nc.sync.dma_start(out=par, in_=p_d[:, :].rearrange("j (h c) -> c j h", h=2))
        epst = sing.tile([128, 1], dt.float32, name="epst")
        nc.vector.memset(epst, EPS)

        def P(j, ch):
            return par[:, PJ[j], ch:ch + 1]

        # ---- persistent big buffers --------------------------------------------
        # xb: [128, 2(k-half), XBW] fp8, DoubleRow-interleaved conv input
        xbt = xbp.tile([128, 2, XBW], dt.float8e4, name="xbt", tag="xb")
        nc.vector.memset(xbt, 0.0)

        # wt: [128(i), 2(k-half), 9(tap), 256(o)] fp8 per conv
        wt = {cv: wtp.tile([128, 2, 9, C], dt.float8e4, name=f"wt{cv}", tag="wt")
              for cv in (1, 2)}

        def cf(name, w=1):
            return stp.tile([128, w], dt.float32, name=name, tag=name)

        # ---- phase 0a: x -> sign(x) into padded fp8 buffer ---------------------
        for im in range(n_img):
            for ch in (0, 1):
                xc = chkp.tile([128, HW], dt.float32, name=f"sx{ch}_{im}", tag="chk")
                nc.sync.dma_start(
                    out=xc,
                    in_=x_d[im, ch * 128:(ch + 1) * 128].rearrange("c h w -> c (h w)"))
                base = MARG + im * SP
                dst = (xbt[:, ch, base:base + SP]
                       .rearrange("p (h w) -> p h w", w=PW)[:, 1:29, 1:29])
                nc.scalar.activation(dst, xc.rearrange("p (h w) -> p h w", w=W),
                                     AF.Sign)

        # ---- phase 0b: weight prep (both convs) --------------------------------
        alpha = {1: cf("alpha1", 2), 2: cf("alpha2", 2)}

        def prep_w(cv, w_d):
            al = alpha[cv]
            for oh in (0, 1):
                wo = wop.tile([128, 2304], dt.bfloat16, name=f"wo{cv}{oh}", tag="wo")
                nc.gpsimd.dma_start(
                    out=wo,
                    in_=w_d[oh * 128:(oh + 1) * 128].rearrange("o i kh kw -> o (i kh kw)"))
                nc.vector.tensor_reduce(al[:, oh:oh + 1], wo, axis=AX.X, op=ALU.add,
                                        apply_absolute_value=True)
                nc.scalar.activation(wo, wo, AF.Sign)
                wos = wo.rearrange("o (i k) -> o i k", k=9)
                for ih in (0, 1):
                    for k9 in range(9):
                        pt = psp.tile([128, 128], dt.bfloat16,
                                      name=f"tp{cv}{oh}{ih}{k9}", tag="ps")
                        nc.tensor.transpose(pt, wos[:, ih * 128:(ih + 1) * 128, k9],
                                            ident)
                        nc.scalar.copy(wt[cv][:, ih, k9, oh * 128:(oh + 1) * 128],
                                       pt)
            nc.vector.tensor_scalar_mul(al, al, 1.0 / 2304.0)

        prep_w(1, w1_d)
        prep_w(2, w2_d)

        # ---- conv macro ---------------------------------------------------------
        # DoubleRow fp8: one matmul contracts both 128-channel halves.
        # Weight-stationary: each (m, tap) weight serves a group of 8 psum
        # banks before switching.
        def conv(cv, S, st):
            tiles = [(im, b) for im in range(n_img) for b in (0, 1)]
            for m in (0, 1):
                for im in range(n_img):
                    S[(m, im)] = actp.tile([128, HW], dt.float32,
                                           name=f"S{cv}_{m}_{im}", tag="act")
                for g0 in range(0, len(tiles), 8):
                    grp = tiles[g0:g0 + 8]
                    pts = {}
                    for (im, b) in grp:
                        pts[(im, b)] = psp.tile([128, 450], dt.float32,
                                                name=f"cp{cv}_{m}_{im}_{b}",
                                                tag="ps")
                    for d in range(9):
                        w_ap = wt[cv][:, :, d, m * 128:(m + 1) * 128]
                        for (im, b) in grp:
                            o = MARG + im * SP + b * 450 + _off(d)
                            nc.tensor.matmul(
                                pts[(im, b)], w_ap, xbt[:, :, o:o + 450],
                                perf_mode=mybir.MatmulPerfMode.DoubleRow,
                                start=(d == 0), stop=(d == 8))
                    for (im, b) in grp:
                        pt = pts[(im, b)]
                        s_t = S[(m, im)]
                        pv = pt.rearrange("p (r c) -> p r c", c=PW)
                        sv = s_t.rearrange("p (r c) -> p r c", c=W)
                        r0 = 1 - b
                        nc.scalar.copy(sv[:, b * 14:(b + 1) * 14, :],
                                       pv[:, r0:r0 + 14, 1:29])
                        if b == 1:
                            for q in (0, 1):
                                nc.vector.bn_stats(st[m][:, im, q],
                                                   s_t[:, q * 392:(q + 1) * 392])

        # ---- per-half stat helpers ---------------------------------------------
        def half_sums(stm, tag):
            # stm: [128, n_img, 2, 6] bn_stats rows -> s2 [128,2] = (sum, sumsq)
            mv = cf(f"mv{tag}", 2)
            nc.vector.bn_aggr(mv, stm.rearrange("p a b s -> p (a b) s"))
            s2 = cf(f"s2{tag}", 2)
            nc.vector.tensor_scalar_mul(s2[:, 0:1], mv[:, 0:1], NLOC)
            t0 = cf(f"t0{tag}")
            nc.vector.tensor_mul(t0, mv[:, 0:1], mv[:, 0:1])
            nc.vector.tensor_add(t0, t0, mv[:, 1:2])
            nc.vector.tensor_scalar_mul(s2[:, 1:2], t0, NLOC)
            return s2

        def ag_reduce(s2, tag):
            # AllGather the per-core [128,2] (sum,sumsq) half-stats; add locally.
            di = dccp.tile([256], dt.float32, name=f"di{tag}", tag=f"di{tag}")
            do = dccp.tile([n_cores * 256], dt.float32, name=f"do{tag}",
                           tag=f"do{tag}")
            nc.sync.dma_start(out=di.rearrange("(c f) -> c f", f=2), in_=s2)
            nc.gpsimd.collective_compute(
                "AllGather", ALU.bypass, replica_groups=rg, ins=[di], outs=[do])
            g8 = cf(f"g8{tag}", 2 * n_cores)
            nc.sync.dma_start(
                out=g8.rearrange("p (f r) -> p f r", f=2),
                in_=do.rearrange("(r c f) -> c f r", c=128, f=2))
            g2 = cf(f"g2{tag}", 2)
            nc.vector.reduce_sum(g2, g8.rearrange("p (f r) -> p f r", f=2),
                                 axis=AX.X)
            return g2

        def mean_var(g2, tag):
            mean = cf(f"mean{tag}")
            var = cf(f"var{tag}")
            msq = cf(f"msq{tag}")
            nc.vector.tensor_scalar_mul(mean, g2[:, 0:1], 1.0 / NTOT)
            nc.vector.tensor_scalar_mul(var, g2[:, 1:2], 1.0 / NTOT)
            nc.vector.tensor_mul(msq, mean, mean)
            nc.vector.tensor_sub(var, var, msq)
            return mean, var

        def inv_of(var, jg, ch, tag):
            # g / sqrt(var + eps)
            sd = cf(f"sd{tag}")
            nc.scalar.activation(sd, var, AF.Sqrt, bias=epst)
            rc = cf(f"rc{tag}")
            nc.vector.reciprocal(rc, sd)
            inv = cf(f"inv{tag}")
            nc.vector.tensor_mul(inv, rc, P(jg, ch))
            return inv

        def bn_conv_coefs(cv, g2, ch, jg, jb, jgam, jbet, tag):
            # y = alpha*S: c=alpha*inv, dg=b-alpha*mean*inv-gamma, A=1-beta, B=beta*c
            mean, var = mean_var(g2, tag)
            al = alpha[cv][:, ch:ch + 1]
            a2 = cf(f"a2{tag}")
            nc.vector.tensor_mul(a2, al, al)
            vy = cf(f"vy{tag}")
            nc.vector.tensor_mul(vy, var, a2)
            inv = inv_of(vy, jg, ch, tag)
            c = cf(f"c{tag}")
            nc.vector.tensor_mul(c, al, inv)
            my = cf(f"my{tag}")
            nc.vector.tensor_mul(my, mean, al)
            nc.vector.tensor_mul(my, my, inv)
            dg = cf(f"dg{tag}")
            nc.vector.tensor_sub(dg, P(jb, ch), my)
            nc.vector.tensor_sub(dg, dg, P(jgam, ch))
            A = cf(f"A{tag}")
            nc.vector.tensor_scalar(A, P(jbet, ch), -1.0, 1.0, ALU.mult, ALU.add)
            B = cf(f"B{tag}")
            nc.vector.tensor_mul(B, P(jbet, ch), c)
            return c, dg, A, B

        def bn_plain_coefs(g2, ch, jg, jb, tag):
            # c = g*inv, d = b - mean*c
            mean, var = mean_var(g2, tag)
            inv = inv_of(var, jg, ch, tag)
            d = cf(f"d{tag}")
            nc.vector.tensor_mul(mean, mean, inv)
            nc.vector.tensor_sub(d, P(jb, ch), mean)
            return inv, d

        # ---- conv1 --------------------------------------------------------------
        st1 = {m: stp.tile([128, n_img, 2, 6], dt.float32, name=f"st1_{m}",
                           tag=f"st1_{m}") for m in (0, 1)}
        S1 = {}
        conv(1, S1, st1)

        # per-half: stats AG -> BN1 coefs -> combine -> h-stats AG -> BN3 -> sign
        sth = {ch: stp.tile([128, n_img, 2, 6], dt.float32, name=f"sth_{ch}",
                            tag=f"sth_{ch}") for ch in (0, 1)}
        HSW = {}
        cc3 = {}
        for ch in (0, 1):
            g2 = ag_reduce(half_sums(st1[ch], f"b1{ch}"), f"b1{ch}")
            c1, d1g, A1, B1 = bn_conv_coefs(1, g2, ch, "g1", "b1", "gamma1",
                                            "beta1", f"b1{ch}")
            # combine: h' = A1*relu(c1*S+d1g) + B1*S + x   (in-place into S)
            for im in range(n_img):
                s_t = S1[(ch, im)]
                xc = chkp.tile([128, HW], dt.float32, name=f"xc{ch}_{im}", tag="chk")
                nc.sync.dma_start(
                    out=xc,
                    in_=x_d[im, ch * 128:(ch + 1) * 128].rearrange("c h w -> c (h w)"))
                t = tmpp.tile([128, HW], dt.float32, name=f"t1_{ch}_{im}", tag="t")
                nc.scalar.activation(t, s_t, AF.Relu, bias=d1g, scale=c1)
                nc.vector.scalar_tensor_tensor(
                    out=xc, in0=s_t, scalar=B1, in1=xc, op0=ALU.mult, op1=ALU.add)
                nc.vector.scalar_tensor_tensor(
                    out=s_t, in0=t, scalar=A1, in1=xc, op0=ALU.mult, op1=ALU.add)
                for q in (0, 1):
                    nc.vector.bn_stats(sth[ch][:, im, q],
                                       s_t[:, q * 392:(q + 1) * 392])
            g2h = ag_reduce(half_sums(sth[ch], f"b3{ch}"), f"b3{ch}")
            cc3[ch] = bn_plain_coefs(g2h, ch, "g3", "b3", f"b3{ch}")

        # BN3 scale + sign + swap, image-major so conv2 can start early
        for im in range(n_img):
            for ch in (0, 1):
                c3, d3 = cc3[ch]
                s_t = S1[(ch, im)]
                nc.vector.tensor_scalar_mul(s_t, s_t, c3)
                base = MARG + im * SP
                dst = (xbt[:, ch, base:base + SP]
                       .rearrange("p (h w) -> p h w", w=PW)[:, 1:29, 1:29])
                nc.scalar.activation(dst, s_t.rearrange("p (h w) -> p h w", w=W),
                                     AF.Sign, bias=d3)
                dr = dswp.tile([128, HW], dt.float32, name=f"hs{ch}_{im}", tag="swap")
                HSW[(ch, im)] = dr
                nc.sync.dma_start(out=dr, in_=s_t)

        # ---- conv2 --------------------------------------------------------------
        st2 = {m: stp.tile([128, n_img, 2, 6], dt.float32, name=f"st2_{m}",
                           tag=f"st2_{m}") for m in (0, 1)}
        S2 = {}
        conv(2, S2, st2)

        # per-half: BN2 coefs -> combine2 -> final-stats AG -> BN4 -> output
        stf = {ch: stp.tile([128, n_img, 2, 6], dt.float32, name=f"stf_{ch}",
                            tag=f"stf_{ch}") for ch in (0, 1)}
        for ch in (0, 1):
            g2 = ag_reduce(half_sums(st2[ch], f"b2{ch}"), f"b2{ch}")
            c2, d2g, A2, B2 = bn_conv_coefs(2, g2, ch, "g2", "b2", "gamma2",
                                            "beta2", f"b2{ch}")
            # combine2: y = A2*relu(c2*S2+d2g) + B2*S2 + hsw   (in-place into S2)
            for im in range(n_img):
                s2t = S2[(ch, im)]
                hc = chkp.tile([128, HW], dt.float32, name=f"hc{ch}_{im}", tag="chk")
                nc.sync.dma_start(out=hc, in_=HSW[(ch, im)])
                t2 = tmpp.tile([128, HW], dt.float32, name=f"t2_{ch}_{im}", tag="t")
                nc.scalar.activation(t2, s2t, AF.Relu, bias=d2g, scale=c2)
                nc.vector.scalar_tensor_tensor(
                    out=hc, in0=s2t, scalar=B2, in1=hc, op0=ALU.mult, op1=ALU.add)
                nc.vector.scalar_tensor_tensor(
                    out=s2t, in0=t2, scalar=A2, in1=hc, op0=ALU.mult, op1=ALU.add)
                for q in (0, 1):
                    nc.vector.bn_stats(stf[ch][:, im, q],
                                       s2t[:, q * 392:(q + 1) * 392])
            g2f = ag_reduce(half_sums(stf[ch], f"b4{ch}"), f"b4{ch}")
            c4, d4 = bn_plain_coefs(g2f, ch, "g4", "b4", f"b4{ch}")
            for im in range(n_img):
                s2t = S2[(ch, im)]
                nc.vector.tensor_scalar(s2t, s2t, c4, d4, ALU.mult, ALU.add)
                nc.sync.dma_start(
                    out=o_d[im, ch * 128:(ch + 1) * 128].rearrange("c h w -> c (h w)"),
                    in_=s2t)

    nc.compile()
    return nc


_NC_CACHE = {}


def get_nc(n_img, n_cores):
    key = (n_img, n_cores)
    if key not in _NC_CACHE:
        _NC_CACHE[key] = build_nc(n_img, n_cores)
    return _NC_CACHE[key]


def pack_pars(inputs):
    return np.stack([np.asarray(inputs[k], np.float32) for k in
                     ["g1", "b1", "g2", "b2", "g3", "b3", "g4", "b4",
                      "gamma1", "beta1", "gamma2", "beta2"]])


def kernel(**inputs):
    from concourse.bass_utils import run_bass_kernel_spmd

    x = np.asarray(inputs["x"], np.float32)
    n_cores = 8
    n_img = x.shape[0] // n_cores
    nc = get_nc(n_img, n_cores)
    pars = pack_pars(inputs)
    w1 = np.asarray(inputs["w1"], np.float32)
    w2 = np.asarray(inputs["w2"], np.float32)
    in_maps = [
        {"x": np.ascontiguousarray(x[c * n_img:(c + 1) * n_img]),
         "w1": w1, "w2": w2, "pars": pars}
        for c in range(n_cores)
    ]
    res = run_bass_kernel_spmd(nc, in_maps, core_ids=list(range(n_cores)))
    return np.concatenate([res.results[c]["out"] for c in range(n_cores)], axis=0)


if __name__ == "__main__":
    nc = build_nc(2, 2)
    print("built ok")


# revision 20
# speedup vs baseline: 1.0730x; 1.0730x over previous
"""Trainium2 Bass kernel for nn_BasicBlock (binary-conv residual block).

Math (reference):
  h  = BN3( RPReLU1(BN1(bconv(sign(x), w1))) + x )
  out= BN4( RPReLU2(BN2(bconv(sign(h), w2))) + h )
with training-mode BN over the FULL batch (exact cross-device stats),
bconv = conv3x3(pad=1) with weights sign(w)*mean(|w|) per out-channel.

Strategy: data-parallel over batch on 8 NeuronCores (16 images/core).
 - fp8e4 +-1 activations/weights; conv = 9 shifted DoubleRow matmuls
   (each contracts both 128-channel halves) into PSUM; integer sums exact.
 - alpha (mean|w|) folded into BN1/BN2 affine; constant per-channel shifts
   absorbed by downstream BNs are dropped.
 - Exact BN stats via per-channel-half (sum,sumsq) AllGather + local add;
   the first half's stats/combine hide under the second half's conv.
 - h' (pre-BN3, scaled by c3) round-trips through DRAM during conv2.
"""

import sys

import numpy as np

sys.path.insert(0, "/opt/trn_rl_repo")

from contextlib import ExitStack

import concourse.bacc as bacc
import concourse.bass as bass
import concourse.mybir as mybir
import concourse.tile as tile
from concourse.masks import make_identity

dt = mybir.dt
AF = mybir.ActivationFunctionType
ALU = mybir.AluOpType
AX = mybir.AxisListType

C = 256
H = W = 28
PH = PW = 30
SP = PH * PW          # padded pixels / image
HW = H * W            # valid pixels / image
MARG = 32             # margin around padded free axis (shifts up to +-31)
EPS = 1e-5
NPAR = 12
PJ = dict(g1=0, b1=1, g2=2, b2=3, g3=4, b3=5, g4=6, b4=7,
          gamma1=8, beta1=9, gamma2=10, beta2=11)


def _off(d):
    kh, kw = d // 3, d % 3
    return (kh - 1) * PW + (kw - 1)


def build_nc(n_img, n_cores):
    nc = bacc.Bacc("TRN2", target_bir_lowering=False, num_devices=n_cores,
                   name="basicblock")
    x_d = nc.declare_dram_parameter("x", [n_img, C, H, W], dt.float32, isOutput=False)
    w1_d = nc.declare_dram_parameter("w1", [C, C, 3, 3], dt.float32, isOutput=False)
    w2_d = nc.declare_dram_parameter("w2", [C, C, 3, 3], dt.float32, isOutput=False)
    p_d = nc.declare_dram_parameter("pars", [NPAR, C], dt.float32, isOutput=False)
    o_d = nc.declare_dram_parameter("out", [n_img, C, H, W], dt.float32, isOutput=True)

    FREE = n_img * SP
    XBW = FREE + 2 * MARG
    NLOC = float(n_img * HW)
    NTOT = float(n_cores * n_img * HW)
    rg = [list(range(n_cores))]

    with ExitStack() as ctx:
        tc = ctx.enter_context(tile.TileContext(nc))
        sing = ctx.enter_context(tc.tile_pool(name="sing", bufs=1))
        xbp = ctx.enter_context(tc.tile_pool(name="xbp", bufs=1))
        wtp = ctx.enter_context(tc.tile_pool(name="wtp", bufs=2))
        wop = ctx.enter_context(tc.tile_pool(name="wop", bufs=2))
        actp = ctx.enter_context(tc.tile_pool(name="actp", bufs=2 * n_img))
        chkp = ctx.enter_context(tc.tile_pool(name="chkp", bufs=4))
        tmpp = ctx.enter_context(tc.tile_pool(name="tmpp", bufs=3))
        stp = ctx.enter_context(tc.tile_pool(name="stp", bufs=1))
        psp = ctx.enter_context(tc.tile_pool(name="psp", bufs=8, space="PSUM"))
        dccp = ctx.enter_context(tc.tile_pool(name="dccp", bufs=1, space="DRAM"))
        dswp = ctx.enter_context(tc.tile_pool(name="dswp", bufs=2 * n_img, space="DRAM"))

        # ---- constants / params -------------------------------------------------
        ident = sing.tile([128, 128], dt.bfloat16, name="ident")
        make_identity(nc, ident)
        par = sing.tile([128, NPAR, 2], dt.float32, name="par")
        nc.sync.dma_start(out=par, in_=p_d[:, :].rearrange("j (h c) -> c j h", h=2))
        epst = sing.tile([128, 1], dt.float32, name="epst")
        nc.vector.memset(epst, EPS)

        def P(j, ch):
            return par[:, PJ[j], ch:ch + 1]

        # ---- persistent big buffers --------------------------------------------
        # xb: [128, 2(k-half), XBW] fp8, DoubleRow-interleaved conv input
        xbt = xbp.tile([128, 2, XBW], dt.float8e4, name="xbt", tag="xb")
        nc.vector.memset(xbt, 0.0)

        # wt: [128(i), 2(k-half), 9(tap), 256(o)] fp8 per conv
        wt = {cv: wtp.tile([128, 2, 9, C], dt.float8e4, name=f"wt{cv}", tag="wt")
              for cv in (1, 2)}

        def cf(name, w=1):
            return stp.tile([128, w], dt.float32, name=name, tag=name)

        # ---- phase 0a: x -> sign(x) into padded fp8 buffer ---------------------
        for im in range(n_img):
            for ch in (0, 1):
                xc = chkp.tile([128, HW], dt.float32, name=f"sx{ch}_{im}", tag="chk")
                nc.sync.dma_start(
                    out=xc,
                    in_=x_d[im, ch * 128:(ch + 1) * 128].rearrange("c h w -> c (h w)"))
                base = MARG + im * SP
                dst = (xbt[:, ch, base:base + SP]
                       .rearrange("p (h w) -> p h w", w=PW)[:, 1:29, 1:29])
                nc.scalar.activation(dst, xc.rearrange("p (h w) -> p h w", w=W),
                                     AF.Sign)

        # ---- phase 0b: weight prep (both convs) --------------------------------
        alpha = {1: cf("alpha1", 2), 2: cf("alpha2", 2)}

        def prep_w(cv, w_d):
            al = alpha[cv]
            for oh in (0, 1):
                wo = wop.tile([128, 2304], dt.bfloat16, name=f"wo{cv}{oh}", tag="wo")
                nc.gpsimd.dma_start(
                    out=wo,
                    in_=w_d[oh * 128:(oh + 1) * 128].rearrange("o i kh kw -> o (i kh kw)"))
                nc.vector.tensor_reduce(al[:, oh:oh + 1], wo, axis=AX.X, op=ALU.add,
                                        apply_absolute_value=True)
                nc.scalar.activation(wo, wo, AF.Sign)
                wos = wo.rearrange("o (i k) -> o i k", k=9)
                for ih in (0, 1):
                    for k9 in range(9):
                        pt = psp.tile([128, 128], dt.bfloat16,
                                      name=f"tp{cv}{oh}{ih}{k9}", tag="ps")
                        nc.tensor.transpose(pt, wos[:, ih * 128:(ih + 1) * 128, k9],
                                            ident)
                        nc.scalar.copy(wt[cv][:, ih, k9, oh * 128:(oh + 1) * 128],
                                       pt)
            nc.vector.tensor_scalar_mul(al, al, 1.0 / 2304.0)

        prep_w(1, w1_d)
        prep_w(2, w2_d)

        # ---- conv macro ---------------------------------------------------------
        # DoubleRow fp8: one matmul contracts both 128-channel halves.
        # Weight-stationary: each (m, tap) weight serves a group of 8 psum
        # banks before switching.
        def conv(cv, S, st):
            tiles = [(im, b) for im in range(n_img) for b in (0, 1)]
            for m in (0, 1):
                for im in range(n_img):
                    S[(m, im)] = actp.tile([128, HW], dt.float32,
                                           name=f"S{cv}_{m}_{im}", tag="act")
                for g0 in range(0, len(tiles), 8):
                    grp = tiles[g0:g0 + 8]
                    pts = {}
                    for (im, b) in grp:
                        pts[(im, b)] = psp.tile([128, 450], dt.float32,
                                                name=f"cp{cv}_{m}_{im}_{b}",
                                                tag="ps")
                    for d in range(9):
                        w_ap = wt[cv][:, :, d, m * 128:(m + 1) * 128]
                        for (im, b) in grp:
                            o = MARG + im * SP + b * 450 + _off(d)
                            nc.tensor.matmul(
                                pts[(im, b)], w_ap, xbt[:, :, o:o + 450],
                                perf_mode=mybir.MatmulPerfMode.DoubleRow,
                                start=(d == 0), stop=(d == 8))
                    for (im, b) in grp:
                        pt = pts[(im, b)]
                        s_t = S[(m, im)]
                        pv = pt.rearrange("p (r c) -> p r c", c=PW)
                        sv = s_t.rearrange("p (r c) -> p r c", c=W)
                        r0 = 1 - b
                        nc.scalar.copy(sv[:, b * 14:(b + 1) * 14, :],
                                       pv[:, r0:r0 + 14, 1:29])
                        if b == 1:
                            for q in (0, 1):
                                nc.vector.bn_stats(st[m][:, im, q],
                                                   s_t[:, q * 392:(q + 1) * 392])

        # ---- per-half stat helpers ---------------------------------------------
        def half_sums(stm, tag):
            # stm: [128, n_img, 2, 6] bn_stats rows -> s2 [128,2] = (sum, sumsq)
            mv = cf(f"mv{tag}", 2)
            nc.vector.bn_aggr(mv, stm.rearrange("p a b s -> p (a b) s"))
            s2 = cf(f"s2{tag}", 2)
            nc.vector.tensor_scalar_mul(s2[:, 0:1], mv[:, 0:1], NLOC)
            t0 = cf(f"t0{tag}")
            nc.vector.tensor_mul(t0, mv[:, 0:1], mv[:, 0:1])
            nc.vector.tensor_add(t0, t0, mv[:, 1:2])
            nc.vector.tensor_scalar_mul(s2[:, 1:2], t0, NLOC)
            return s2

        def ag_reduce(s2, tag):
            # AllGather the per-core [128,2] (sum,sumsq) half-stats; add locally.
            di = dccp.tile([256], dt.float32, name=f"di{tag}", tag=f"di{tag}")
            do = dccp.tile([n_cores * 256], dt.float32, name=f"do{tag}",
                           tag=f"do{tag}")
            nc.sync.dma_start(out=di.rearrange("(c f) -> c f", f=2), in_=s2)
            nc.gpsimd.collective_compute(
                "AllGather", ALU.bypass, replica_groups=rg, ins=[di], outs=[do])
            g8 = cf(f"g8{tag}", 2 * n_cores)
            nc.sync.dma_start(
                out=g8.rearrange("p (f r) -> p f r", f=2),
                in_=do.rearrange("(r c f) -> c f r", c=128, f=2))
            g2 = cf(f"g2{tag}", 2)
            nc.vector.reduce_sum(g2, g8.rearrange("p (f r) -> p f r", f=2),
                                 axis=AX.X)
            return g2

        def mean_var(g2, tag):
            mean = cf(f"mean{tag}")
            var = cf(f"var{tag}")
            msq = cf(f"msq{tag}")
            nc.vector.tensor_scalar_mul(mean, g2[:, 0:1], 1.0 / NTOT)
            nc.vector.tensor_scalar_mul(var, g2[:, 1:2], 1.0 / NTOT)
            nc.vector.tensor_mul(msq, mean, mean)
            nc.vector.tensor_sub(var, var, msq)
            return mean, var

        def inv_of(var, jg, ch, tag):
            # g / sqrt(var + eps)
            sd = cf(f"sd{tag}")
            nc.scalar.activation(sd, var, AF.Sqrt, bias=epst)
            rc = cf(f"rc{tag}")
            nc.vector.reciprocal(rc, sd)
            inv = cf(f"inv{tag}")
            nc.vector.tensor_mul(inv, rc, P(jg, ch))
            return inv

        def bn_conv_coefs(cv, g2, ch, jg, jb, jgam, tag):
            # y = alpha*S: c=alpha*inv, dg=b-alpha*mean*inv-gamma
            mean, var = mean_var(g2, tag)
            al = alpha[cv][:, ch:ch + 1]
            a2 = cf(f"a2{tag}")
            nc.vector.tensor_mul(a2, al, al)
            vy = cf(f"vy{tag}")
            nc.vector.tensor_mul(vy, var, a2)
            inv = inv_of(vy, jg, ch, tag)
            c = cf(f"c{tag}")
            nc.vector.tensor_mul(c, al, inv)
            my = cf(f"my{tag}")
            nc.vector.tensor_mul(my, mean, al)
            nc.vector.tensor_mul(my, my, inv)
            dg = cf(f"dg{tag}")
            nc.vector.tensor_sub(dg, P(jb, ch), my)
            nc.vector.tensor_sub(dg, dg, P(jgam, ch))
            return c, dg

        def bn_plain_coefs(g2, ch, jg, jb, tag):
            # c = g*inv, d = b - mean*c
            mean, var = mean_var(g2, tag)
            inv = inv_of(var, jg, ch, tag)
            d = cf(f"d{tag}")
            nc.vector.tensor_mul(mean, mean, inv)
            nc.vector.tensor_sub(d, P(jb, ch), mean)
            return inv, d

        # ---- warmup collective (first cc pays ~20us init; hide it here) --------
        diw = dccp.tile([256], dt.float32, name="diw", tag="diw")
        dow = dccp.tile([n_cores * 256], dt.float32, name="dow", tag="dow")
        nc.sync.dma_start(out=diw.rearrange("(c f) -> c f", f=2), in_=par[:, 0, :])
        nc.gpsimd.collective_compute(
            "AllGather", ALU.bypass, replica_groups=rg, ins=[diw], outs=[dow])
        warmt = cf("warmt", 2)
        nc.sync.dma_start(out=warmt, in_=dow.rearrange("(r x) -> x r",
                                                       x=256)[0:128, 0:2])

        # ---- conv1 --------------------------------------------------------------
        st1 = {m: stp.tile([128, n_img, 2, 6], dt.float32, name=f"st1_{m}",
                           tag=f"st1_{m}") for m in (0, 1)}
        S1 = {}
        conv(1, S1, st1)

        # stat AGs for both halves first: collectives fire in emission order,
        # and ch0's must not queue behind anything that needs ch1's conv half.
        g2b1 = {ch: ag_reduce(half_sums(st1[ch], f"b1{ch}"), f"b1{ch}")
                for ch in (0, 1)}
        cc1 = {ch: bn_conv_coefs(1, g2b1[ch], ch, "g1", "b1", "gamma1", f"b1{ch}")
               for ch in (0, 1)}

        # per-half: combine -> h-stats AG -> BN3 coefs
        sth = {ch: stp.tile([128, n_img, 2, 6], dt.float32, name=f"sth_{ch}",
                            tag=f"sth_{ch}") for ch in (0, 1)}
        HSW = {}
        cc3 = {}
        for ch in (0, 1):
            c1, d1g = cc1[ch]
            # combine: h' = prelu(c1*S+d1g; beta1) + x   (in-place into S)
            for im in range(n_img):
                s_t = S1[(ch, im)]
                xc = chkp.tile([128, HW], dt.float32, name=f"xc{ch}_{im}", tag="chk")
                nc.sync.dma_start(
                    out=xc,
                    in_=x_d[im, ch * 128:(ch + 1) * 128].rearrange("c h w -> c (h w)"))
                t = tmpp.tile([128, HW], dt.float32, name=f"t1_{ch}_{im}", tag="t")
                nc.scalar.activation(t, s_t, AF.Prelu, bias=d1g, scale=c1,
                                     alpha=P("beta1", ch))
                nc.vector.tensor_add(s_t, t, xc)
                for q in (0, 1):
                    nc.vector.bn_stats(sth[ch][:, im, q],
                                       s_t[:, q * 392:(q + 1) * 392])
            g2h = ag_reduce(half_sums(sth[ch], f"b3{ch}"), f"b3{ch}")
            cc3[ch] = bn_plain_coefs(g2h, ch, "g3", "b3", f"b3{ch}")

        # BN3 scale + sign + swap, image-major so conv2 can start early
        for im in range(n_img):
            for ch in (0, 1):
                c3, d3 = cc3[ch]
                s_t = S1[(ch, im)]
                nc.vector.tensor_scalar_mul(s_t, s_t, c3)
                base = MARG + im * SP
                dst = (xbt[:, ch, base:base + SP]
                       .rearrange("p (h w) -> p h w", w=PW)[:, 1:29, 1:29])
                nc.scalar.activation(dst, s_t.rearrange("p (h w) -> p h w", w=W),
                                     AF.Sign, bias=d3)
                dr = dswp.tile([128, HW], dt.float32, name=f"hs{ch}_{im}", tag="swap")
                HSW[(ch, im)] = dr
                nc.sync.dma_start(out=dr, in_=s_t)

        # ---- conv2 --------------------------------------------------------------
        st2 = {m: stp.tile([128, n_img, 2, 6], dt.float32, name=f"st2_{m}",
                           tag=f"st2_{m}") for m in (0, 1)}
        S2 = {}
        conv(2, S2, st2)

        # per-half: BN2 coefs -> combine2 -> final-stats AG -> BN4 -> output
        g2b2 = {ch: ag_reduce(half_sums(st2[ch], f"b2{ch}"), f"b2{ch}")
                for ch in (0, 1)}
        cc2 = {ch: bn_conv_coefs(2, g2b2[ch], ch, "g2", "b2", "gamma2", f"b2{ch}")
               for ch in (0, 1)}
        stf = {ch: stp.tile([128, n_img, 2, 6], dt.float32, name=f"stf_{ch}",
                            tag=f"stf_{ch}") for ch in (0, 1)}
        for ch in (0, 1):
            c2, d2g = cc2[ch]
            # combine2: y = prelu(c2*S2+d2g; beta2) + hsw   (in-place into S2)
            for im in range(n_img):
                s2t = S2[(ch, im)]
                hc = chkp.tile([128, HW], dt.float32, name=f"hc{ch}_{im}", tag="chk")
                nc.sync.dma_start(out=hc, in_=HSW[(ch, im)])
                t2 = tmpp.tile([128, HW], dt.float32, name=f"t2_{ch}_{im}", tag="t")
                nc.scalar.activation(t2, s2t, AF.Prelu, bias=d2g, scale=c2,
                                     alpha=P("beta2", ch))
                nc.vector.tensor_add(s2t, t2, hc)
                for q in (0, 1):
                    nc.vector.bn_stats(stf[ch][:, im, q],
                                       s2t[:, q * 392:(q + 1) * 392])
            g2f = ag_reduce(half_sums(stf[ch], f"b4{ch}"), f"b4{ch}")
            c4, d4 = bn_plain_coefs(g2f, ch, "g4", "b4", f"b4{ch}")
            for im in range(n_img):
                s2t = S2[(ch, im)]
                nc.vector.tensor_scalar(s2t, s2t, c4, d4, ALU.mult, ALU.add)
                nc.sync.dma_start(
                    out=o_d[im, ch * 128:(ch + 1) * 128].rearrange("c h w -> c (h w)"),
                    in_=s2t)

    nc.compile()
    return nc


_NC_CACHE = {}


def get_nc(n_img, n_cores):
    key = (n_img, n_cores)
    if key not in _NC_CACHE:
        _NC_CACHE[key] = build_nc(n_img, n_cores)
    return _NC_CACHE[key]


def pack_pars(inputs):
    return np.stack([np.asarray(inputs[k], np.float32) for k in
                     ["g1", "b1", "g2", "b2", "g3", "b3", "g4", "b4",
                      "gamma1", "beta1", "gamma2", "beta2"]])


def kernel(**inputs):
    from concourse.bass_utils import run_bass_kernel_spmd

    x = np.asarray(inputs["x"], np.float32)
    n_cores = 8
    n_img = x.shape[0] // n_cores
    nc = get_nc(n_img, n_cores)
    pars = pack_pars(inputs)
    w1 = np.asarray(inputs["w1"], np.float32)
    w2 = np.asarray(inputs["w2"], np.float32)
    in_maps = [
        {"x": np.ascontiguousarray(x[c * n_img:(c + 1) * n_img]),
         "w1": w1, "w2": w2, "pars": pars}
        for c in range(n_cores)
    ]
    res = run_bass_kernel_spmd(nc, in_maps, core_ids=list(range(n_cores)))
    return np.concatenate([res.results[c]["out"] for c in range(n_cores)], axis=0)


if __name__ == "__main__":
    nc = build_nc(2, 2)
    print("built ok")
